# revision 1
# baseline (speedup 1.0000x reference)
"""Trainium2 Bass kernel for banded local attention (kernel_size=128).

Problem: x[4,4096,512]; q = x@Wq.T+bq, k = x@Wk.T+bk (H=512);
scores = q@k.T masked to |i-j|<128; softmax; out = attn @ x.

Sharding: 8 cores = 4 batches x 2 sequence halves (2048 queries each) with a
128-row halo of keys on each side (2304 local key rows, zero padded at the
global sequence edges). For the h=1 half the sequence is passed REVERSED so
the padded/invalid key region is always local rows [0,128) and the edge mask
is only needed for query block 0 -> all 8 cores run the identical program
(pure SPMD, no collectives). Host un-reverses the h=1 outputs.

Per-core data layout (all fp32, matmuls in float32r = full-rate fp22):
  xT   [512, 2304]  x_halo transposed (d on partitions) - rhs/lhsT for projs
  xrow [2304, 512]  x_halo row-major - rhs ("values") for attn @ x
  wqT/wkT [512,512] weight transposed [d, h] - lhsT for projections
  bq/bk [512]       biases (added via ACT Identity during PSUM->SBUF copy)
  masks [2,128,384] additive band masks (0 / -1e30); slot 1 = edge variant
On chip:
  qT [h,2048] = wqT.T @ xT (+bq)  4 h-tiles; serves as lhsT for scores
  kT [h,2304] = wkT.T @ xT (+bk)  4 h-tiles; serves as rhs for scores
  per 128-query block qb: s[128,384] = qT_blk.T @ kT_window (PSUM),
  s += mask (DVE), rowmax m (DVE), p = exp(s - m) with accumulated row
  sum l (ACT), pT = PE-transpose(p), out = pT.T @ xrow_window, scaled by
  1/l during the PSUM->SBUF copy (ACT, scale AP).
"""
import sys

if "/opt/trn_rl_repo" not in sys.path:
    sys.path.insert(0, "/opt/trn_rl_repo")

import numpy as np

B, S, D, H = 4, 4096, 512, 512
KS = 128
HALF = S // 2            # 2048 queries per core
HALO = KS                # 128
SK = HALF + 2 * HALO     # 2304 local key rows
WIN = 3 * 128            # 384-wide key window per query block
NBLK = HALF // 128       # 16 query blocks
NEG = -1e30
N_CORES = 8

F32 = None  # set after import
_cached = {}


def _build_program():
    import concourse.bass as bass
    import concourse.tile as tile
    import concourse.mybir as mybir
    from concourse import bacc

    f32 = mybir.dt.float32
    f32r = mybir.dt.float32r
    AF = mybir.ActivationFunctionType
    AX = mybir.AxisListType

    nc = bacc.Bacc("TRN2", target_bir_lowering=False, debug=False,
                   num_devices=N_CORES)

    xT_d = nc.dram_tensor("xT", [D, SK], f32r, kind="ExternalInput").ap()
    xrow_d = nc.dram_tensor("xrow", [SK, D], f32r, kind="ExternalInput").ap()
    wqT_d = nc.dram_tensor("wqT", [D, H], f32r, kind="ExternalInput").ap()
    wkT_d = nc.dram_tensor("wkT", [D, H], f32r, kind="ExternalInput").ap()
    bq_d = nc.dram_tensor("bq", [H, 1], f32, kind="ExternalInput").ap()
    bk_d = nc.dram_tensor("bk", [H, 1], f32, kind="ExternalInput").ap()
    masks_d = nc.dram_tensor("masks", [2, 128, WIN], f32,
                             kind="ExternalInput").ap()
    out_d = nc.dram_tensor("out", [HALF, D], f32, kind="ExternalOutput").ap()

    DT = D // 128   # 4 d-tiles
    HT = H // 128   # 4 h-tiles
    JT = SK // 128  # 18 key row tiles

    with tile.TileContext(nc) as tc:
        with (
            tc.tile_pool(name="big", bufs=1) as big,
            tc.tile_pool(name="work", bufs=3) as work,
            tc.tile_pool(name="stat", bufs=4) as stat,
            tc.tile_pool(name="psA", bufs=2, space="PSUM") as psA,
            tc.tile_pool(name="psB", bufs=2, space="PSUM") as psB,
        ):
            # ---- resident inputs ----
            wq = [big.tile([128, H], f32r, tag=f"wq{t}", name=f"wq{t}") for t in range(DT)]
            wk = [big.tile([128, H], f32r, tag=f"wk{t}", name=f"wk{t}") for t in range(DT)]
            for t in range(DT):
                nc.sync.dma_start(wq[t], wqT_d[t * 128:(t + 1) * 128, :])
                nc.sync.dma_start(wk[t], wkT_d[t * 128:(t + 1) * 128, :])
            bq = [big.tile([128, 1], f32, tag=f"bq{t}", name=f"bq{t}") for t in range(HT)]
            bk = [big.tile([128, 1], f32, tag=f"bk{t}", name=f"bk{t}") for t in range(HT)]
            for t in range(HT):
                nc.sync.dma_start(bq[t], bq_d[t * 128:(t + 1) * 128, :])
                nc.sync.dma_start(bk[t], bk_d[t * 128:(t + 1) * 128, :])
            masks = [big.tile([128, WIN], f32, tag=f"mask{i}", name=f"maskt{i}") for i in range(2)]
            for i in range(2):
                nc.sync.dma_start(masks[i], masks_d[i])
            xT = [big.tile([128, SK], f32r, tag=f"xT{t}", name=f"xT{t}") for t in range(DT)]
            for t in range(DT):
                nc.sync.dma_start(xT[t], xT_d[t * 128:(t + 1) * 128, :])

            # ---- projections: qT[h,i] and kT[h,j] ----
            qT = [big.tile([128, HALF], f32r, tag=f"qT{t}", name=f"qT{t}") for t in range(HT)]
            kT = [big.tile([128, SK], f32r, tag=f"kT{t}", name=f"kT{t}") for t in range(HT)]

            def project(dst, w_tiles, bias, n_cols, off=0, on_dve=False):
                # dst[ht][h, c] = sum_d w[d, h] * xT[d, c] + bias[h]
                chunks = []
                c0 = 0
                while c0 < n_cols:
                    cw = min(512, n_cols - c0)
                    chunks.append((c0, cw))
                    c0 += cw
                for ht in range(HT):
                    for (c0, cw) in chunks:
                        ps = psA.tile([128, 512], f32, tag="proj")
                        for dt_i in range(DT):
                            nc.tensor.matmul(
                                ps[:, :cw],
                                lhsT=w_tiles[dt_i][:, ht * 128:(ht + 1) * 128],
                                rhs=xT[dt_i][:, off + c0:off + c0 + cw],
                                start=(dt_i == 0),
                                stop=(dt_i == DT - 1),
                            )
                        if on_dve:
                            nc.vector.tensor_scalar_add(
                                dst[ht][:, c0:c0 + cw], ps[:, :cw], bias[ht])
                        else:
                            nc.scalar.activation(
                                dst[ht][:, c0:c0 + cw], ps[:, :cw],
                                AF.Identity, bias=bias[ht], scale=1.0,
                            )

            project(qT, wq, bq, HALF, off=HALO, on_dve=True)
            project(kT, wk, bk, SK)

            # xrow ("values") DMAs issued after the projection instructions so
            # the Sync queue services weights/xT first; these 4.7MB stream in
            # while the PE is busy with the projections.
            xrow = [big.tile([128, D], f32r, tag=f"xr{j}", name=f"xr{j}")
                    for j in range(JT)]
            for j in range(JT):
                nc.sync.dma_start(xrow[j], xrow_d[j * 128:(j + 1) * 128, :])

            # ---- identity for PE transpose ----
            ident = big.tile([128, 128], f32, tag="ident")
            from concourse.masks import make_identity
            make_identity(nc, ident)

            # ---- per query-block attention ----
            for qb in range(NBLK):
                j0 = qb * 128  # key window start (local row / kT col)
                mask = masks[1] if qb == 0 else masks[0]

                s_ps = psA.tile([128, WIN], f32, tag="s")
                for ht in range(HT):
                    nc.tensor.matmul(
                        s_ps,
                        lhsT=qT[ht][:, qb * 128:(qb + 1) * 128],
                        rhs=kT[ht][:, j0:j0 + WIN],
                        start=(ht == 0),
                        stop=(ht == HT - 1),
                    )
                # s += mask ; rowmax ; p = exp(s - m), l = rowsum(p)
                s_sb = work.tile([128, WIN], f32, tag="s_sb")
                nc.vector.tensor_add(s_sb, s_ps, mask)
                m = stat.tile([128, 1], f32, tag="m")
                nc.vector.reduce_max(m, s_sb, axis=AX.X)
                negm = stat.tile([128, 1], f32, tag="negm")
                nc.scalar.mul(negm, m, -1.0)
                p_sb = work.tile([128, WIN], f32, tag="p_sb")
                lsum = stat.tile([128, 1], f32, tag="lsum")
                nc.scalar.activation(p_sb, s_sb, AF.Exp, bias=negm,
                                     scale=1.0, accum_out=lsum)
                rinv = stat.tile([128, 1], f32, tag="rinv")
                nc.vector.reciprocal(rinv, lsum)

                # transpose p -> pT (3 x [128,128])
                pT_ps = psB.tile([128, 3, 128], f32, tag="pT")
                for jt in range(3):
                    nc.tensor.transpose(
                        pT_ps[:, jt, :],
                        p_sb[:, jt * 128:(jt + 1) * 128],
                        ident,
                    )
                pT_sb = work.tile([128, 3, 128], f32r, tag="pT_sb")
                nc.vector.tensor_copy(pT_sb, pT_ps)

                # out_blk[i, d] = sum_j p[i, j] * xrow[j, d], scaled by 1/l
                o_ps = psB.tile([128, D], f32, tag="o")
                for jt in range(3):
                    nc.tensor.matmul(
                        o_ps,
                        lhsT=pT_sb[:, jt, :],
                        rhs=xrow[qb + jt],
                        start=(jt == 0),
                        stop=(jt == 2),
                    )
                o_sb = work.tile([128, D], f32, tag="o_sb")
                nc.scalar.activation(o_sb, o_ps, AF.Identity,
                                     bias=0.0, scale=rinv)
                nc.sync.dma_start(out_d[qb * 128:(qb + 1) * 128, :], o_sb)

    nc.compile()
    return nc


def _get_program():
    if "nc" not in _cached:
        _cached["nc"] = _build_program()
    return _cached["nc"]


def _make_masks():
    a = np.arange(128)[:, None]
    y = np.arange(WIN)[None, :]
    band = (y - a >= 1) & (y - a <= 255)
    base = np.where(band, 0.0, NEG).astype(np.float32)
    edge = np.where(band & (y >= 128), 0.0, NEG).astype(np.float32)
    return np.stack([base, edge])


def kernel(x, Wq_w, Wq_b, Wk_w, Wk_b, _trace=False):
    from concourse.bass_utils import run_bass_kernel_spmd

    x = np.ascontiguousarray(np.asarray(x, np.float32))
    wqT = np.ascontiguousarray(np.asarray(Wq_w, np.float32).T)
    wkT = np.ascontiguousarray(np.asarray(Wk_w, np.float32).T)
    bq = np.ascontiguousarray(np.asarray(Wq_b, np.float32))
    bk = np.ascontiguousarray(np.asarray(Wk_b, np.float32))
    masks = _make_masks()

    nc = _get_program()

    in_maps = []
    for core in range(N_CORES):
        b, h = divmod(core, 2)
        x_halo = np.zeros((SK, D), np.float32)
        if h == 0:
            x_halo[HALO:] = x[b, 0:HALF + HALO]
        else:
            x_halo[HALO:] = x[b, S - HALF - HALO:][::-1]
        in_maps.append({
            "xT": np.ascontiguousarray(x_halo.T),
            "xrow": x_halo,
            "wqT": wqT,
            "wkT": wkT,
            "bq": bq.reshape(H, 1),
            "bk": bk.reshape(H, 1),
            "masks": masks,
        })

    res = run_bass_kernel_spmd(nc, in_maps, core_ids=list(range(N_CORES)),
                               trace=_trace)
    _cached["last_result"] = res

    y = np.zeros((B, S, D), np.float32)
    for core in range(N_CORES):
        b, h = divmod(core, 2)
        o = res.results[core]["out"]
        if h == 0:
            y[b, :HALF] = o
        else:
            y[b, HALF:] = o[::-1]
    return y



# revision 12
# speedup vs baseline: 1.0475x; 1.0475x over previous
"""Trainium2 Bass kernel for banded local attention (kernel_size=128).

Problem: x[4,4096,512]; q = x@Wq.T+bq, k = x@Wk.T+bk (H=512);
scores = q@k.T masked to |i-j|<128; softmax; out = attn @ x.

Sharding: 8 cores = 4 batches x 2 sequence halves (2048 queries each) with a
128-row halo of keys on each side (2304 local key rows, zero padded at the
global sequence edges). For the h=1 half the sequence is passed REVERSED so
the padded/invalid key region is always local rows [0,128) and the edge mask
is only needed for query block 0 -> all 8 cores run the identical program
(pure SPMD, no collectives). Host un-reverses the h=1 outputs.

Precision: the score spread (sigma ~ 22) makes the softmax near-argmax, so
q/k/x stay f32r (fp22) for the scores; p / values / out are bf16 (errors in
p cancel through the l normalizer; bf16 V adds ~0.2% per element).

v3 datapath (vs v0 baseline at 110.8us):
  - band masks generated on-chip (gpsimd affine_select) in bf16 and added
    into the accumulating scores PSUM group via an identity-weighted matmul
    (one extra 384-cycle PE op per block instead of a 744ns DVE add)
  - p = exp(s - m) written bf16 by ACT with the row-sum l from accum_out;
    invalid positions are exactly 0 (masked to -1e30 in PSUM)
  - PE transposes run on bf16 p (1 cyc/row instead of f32's 2)
  - values x loaded once as bf16 [p, kt, d] (half the bytes of the f32 xrow
    copy the baseline used); out stored bf16, host widens
  - DMA issue order: wk by h-tile, xT chunk 0, rest of wk/xT interleaved, so
    the first k-projection matmul starts ~6us in (baseline: 25.7us); the
    k-projection runs chunk-by-chunk as columns land, q-projection pieces
    interleave with attention blocks to overlap ACT/DVE softmax work with
    PE projection matmuls
"""
import sys

if "/opt/trn_rl_repo" not in sys.path:
    sys.path.insert(0, "/opt/trn_rl_repo")

import numpy as np

B, S, D, H = 4, 4096, 512, 512
KS = 128
HALF = S // 2            # 2048 queries per core
HALO = KS                # 128
SK = HALF + 2 * HALO     # 2304 local key rows
KT = SK // 128           # 18 key row tiles
WIN = 3 * 128            # 384-wide key window per query block
NBLK = HALF // 128       # 16 query blocks
NEG = -1e30
N_CORES = 8

DT = D // 128   # 4 d-tiles
HT = H // 128   # 4 h-tiles

# k-projection chunks over SK columns (equal to xT chunk tiles)
K_CHUNKS = [(0, 512), (512, 512), (1024, 512), (1536, 512), (2048, 256)]
# q-projection pieces over HALF output cols; xT col = out col + HALO.
# (out_c0, width, xT_chunk_idx, offset_within_chunk)
Q_PIECES = [(0, 384, 0, 128), (384, 512, 1, 0), (896, 512, 2, 0),
            (1408, 512, 3, 0), (1920, 128, 4, 0)]
# attention blocks emitted after each q piece completes
Q_BLOCKS = [[0, 1, 2], [3, 4, 5, 6], [7, 8, 9, 10], [11, 12, 13, 14], [15]]

_cached = {}


def _build_program():
    import concourse.bass as bass
    import concourse.tile as tile
    import concourse.mybir as mybir
    from concourse import bacc
    from concourse.masks import make_identity

    f32 = mybir.dt.float32
    f32r = mybir.dt.float32r
    bf16 = mybir.dt.bfloat16
    AF = mybir.ActivationFunctionType
    AX = mybir.AxisListType

    nc = bacc.Bacc("TRN2", target_bir_lowering=False, debug=False,
                   num_devices=N_CORES)

    # dram inputs (host pre-arranged; see kernel())
    xT_d = nc.dram_tensor("xTp", [128, DT, SK], f32r,
                          kind="ExternalInput").ap()
    wq_d = nc.dram_tensor("wqp", [128, DT, H], f32r,
                          kind="ExternalInput").ap()
    wk_d = nc.dram_tensor("wkp", [128, DT, H], f32r,
                          kind="ExternalInput").ap()
    bias_d = nc.dram_tensor("biasp", [128, 8], f32, kind="ExternalInput").ap()
    xrow_d = nc.dram_tensor("xrowp", [128, KT, D], bf16,
                            kind="ExternalInput").ap()
    out_d = nc.dram_tensor("out", [HALF, D], bf16, kind="ExternalOutput").ap()

    with tile.TileContext(nc) as tc:
        with (
            tc.tile_pool(name="big", bufs=1) as big,
            tc.tile_pool(name="work", bufs=3) as work,
            tc.tile_pool(name="stat", bufs=4) as stat,
            tc.tile_pool(name="psP", bufs=2, space="PSUM") as psP,
            tc.tile_pool(name="psS", bufs=2, space="PSUM") as psS,
            tc.tile_pool(name="psT", bufs=2, space="PSUM") as psT,
            tc.tile_pool(name="psO", bufs=2, space="PSUM") as psO,
        ):
            # ---- input DMAs, ordered so k-proj chunk 0 starts earliest ----
            wk = big.tile([128, DT, H], f32r, tag="wk", name="wk")
            nc.sync.dma_start(wk[:, :, 0:128], wk_d[:, :, 0:128])
            xTc = [big.tile([128, DT, cw], f32r, tag=f"xT{c}", name=f"xT{c}")
                   for c, (c0, cw) in enumerate(K_CHUNKS)]
            c0, cw = K_CHUNKS[0]
            nc.sync.dma_start(xTc[0], xT_d[:, :, c0:c0 + cw])
            for ht in range(1, HT):
                nc.sync.dma_start(wk[:, :, ht * 128:(ht + 1) * 128],
                                  wk_d[:, :, ht * 128:(ht + 1) * 128])
            c0, cw = K_CHUNKS[1]
            nc.sync.dma_start(xTc[1], xT_d[:, :, c0:c0 + cw])
            wq = big.tile([128, DT, H], f32r, tag="wq", name="wq")
            nc.sync.dma_start(wq, wq_d)
            for c in (2, 3, 4):
                c0, cw = K_CHUNKS[c]
                nc.sync.dma_start(xTc[c], xT_d[:, :, c0:c0 + cw])
            biasp = big.tile([128, 8], f32, tag="biasp", name="biasp")
            nc.sync.dma_start(biasp, bias_d)
            xrowp = big.tile([128, KT, D], bf16, tag="xrowp", name="xrowp")
            nc.sync.dma_start(xrowp, xrow_d)

            # ---- on-chip constants (GpSimd; overlaps the DMAs) ----
            ident = big.tile([128, 128], bf16, tag="ident")
            make_identity(nc, ident)
            # additive band masks over the 384-wide window: tile 0 valid iff
            # y > a, tile 1 always valid, tile 2 valid iff y < a; the edge
            # variant (block 0) masks all of tile 0 (padded halo rows)
            masks = []
            for mi in range(2):
                mk = big.tile([128, WIN], bf16, tag=f"mask{mi}")
                nc.gpsimd.memset(mk, 0.0)
                if mi == 0:
                    nc.gpsimd.affine_select(
                        out=mk[:, 0:128], in_=mk[:, 0:128],
                        compare_op=mybir.AluOpType.is_ge, fill=NEG,
                        base=-1, pattern=[[1, 128]], channel_multiplier=-1)
                else:
                    nc.gpsimd.memset(mk[:, 0:128], NEG)
                nc.gpsimd.affine_select(
                    out=mk[:, 256:384], in_=mk[:, 256:384],
                    compare_op=mybir.AluOpType.is_ge, fill=NEG,
                    base=-1, pattern=[[-1, 128]], channel_multiplier=1)
                masks.append(mk)

            # ---- projections ----
            kT = [big.tile([128, SK], f32r, tag=f"kT{t}", name=f"kT{t}")
                  for t in range(HT)]
            qT = [big.tile([128, HALF], f32r, tag=f"qT{t}", name=f"qT{t}")
                  for t in range(HT)]

            ncopy = [0]

            def proj_chunk(dst, w, bias_col, c0, cw, xc, xoff):
                # dst[ht][h, c0:c0+cw] = sum_d w[d,h] xT[d, xoff:xoff+cw] + b
                for ht in range(HT):
                    ps = psP.tile([128, 512], f32, tag="proj")
                    for dt_i in range(DT):
                        nc.tensor.matmul(
                            ps[:, :cw],
                            lhsT=w[:, dt_i, ht * 128:(ht + 1) * 128],
                            rhs=xTc[xc][:, dt_i, xoff:xoff + cw],
                            start=(dt_i == 0),
                            stop=(dt_i == DT - 1),
                        )
                    # alternate the bias-add copy between ACT and DVE
                    bc = bias_col + ht
                    if ncopy[0] % 2 == 0:
                        nc.scalar.activation(
                            dst[ht][:, c0:c0 + cw], ps[:, :cw],
                            AF.Identity, bias=biasp[:, bc:bc + 1],
                            scale=1.0)
                    else:
                        nc.vector.tensor_scalar_add(
                            dst[ht][:, c0:c0 + cw], ps[:, :cw],
                            biasp[:, bc:bc + 1])
                    ncopy[0] += 1

            # k projection over all 5 chunks
            for c, (c0, cw) in enumerate(K_CHUNKS):
                proj_chunk(kT, wk, 4, c0, cw, c, 0)

            # ---- per query-block attention ----
            def attention(qb):
                j0 = qb * 128
                s_ps = psS.tile([128, WIN], f32, tag="s")
                for ht in range(HT):
                    nc.tensor.matmul(
                        s_ps,
                        lhsT=qT[ht][:, qb * 128:(qb + 1) * 128],
                        rhs=kT[ht][:, j0:j0 + WIN],
                        start=(ht == 0),
                        stop=False,
                    )
                # add the band mask into the accumulating PSUM group with an
                # identity-weighted matmul: s[i,y] += mask[i,y]
                nc.tensor.matmul(
                    s_ps,
                    lhsT=ident,
                    rhs=masks[1] if qb == 0 else masks[0],
                    start=False,
                    stop=True,
                )
                negm = stat.tile([128, 1], f32, tag="negm")
                nc.vector.reduce_max(negm, s_ps, axis=AX.X, negate=True)
                p = work.tile([128, WIN], bf16, tag="p")
                lsum = stat.tile([128, 1], f32, tag="lsum")
                nc.scalar.activation(p, s_ps, AF.Exp, bias=negm, scale=1.0,
                                     accum_out=lsum)
                rinv = stat.tile([128, 1], f32, tag="rinv")
                nc.vector.reciprocal(rinv, lsum)

                # transpose p (bf16, 1 cyc/row); ACT copies PSUM -> SBUF
                pT_ps = psT.tile([128, 3, 128], bf16, tag="pT")
                for jt in range(3):
                    nc.tensor.transpose(
                        pT_ps[:, jt, :], p[:, jt * 128:(jt + 1) * 128], ident)
                pT = work.tile([128, 3, 128], bf16, tag="pTs")
                nc.scalar.activation(pT, pT_ps, AF.Identity, bias=0.0,
                                     scale=1.0)

                # out_blk[i, d] = sum_j p[i, j] xrow[j, d]
                o_ps = psO.tile([128, D], f32, tag="o")
                for jt in range(3):
                    nc.tensor.matmul(o_ps, lhsT=pT[:, jt, :],
                                     rhs=xrowp[:, qb + jt, :],
                                     start=(jt == 0), stop=(jt == 2))
                o_sb = work.tile([128, D], bf16, tag="o_sb")
                nc.vector.tensor_scalar_mul(o_sb, o_ps, rinv)
                nc.sync.dma_start(out_d[qb * 128:(qb + 1) * 128, :], o_sb)

            # q projection piece by piece, attention interleaved
            for (c0, cw, xc, xoff), blocks in zip(Q_PIECES, Q_BLOCKS):
                proj_chunk(qT, wq, 0, c0, cw, xc, xoff)
                for qb in blocks:
                    attention(qb)

    nc.compile()
    return nc


def _get_program():
    if "nc" not in _cached:
        _cached["nc"] = _build_program()
    return _cached["nc"]


def kernel(x, Wq_w, Wq_b, Wk_w, Wk_b, _trace=False):
    import ml_dtypes
    from concourse.bass_utils import run_bass_kernel_spmd

    bf16 = ml_dtypes.bfloat16

    x = np.ascontiguousarray(np.asarray(x, np.float32))
    # weights pre-transposed to [D, H] then packed [p, dt, h]
    wqp = np.ascontiguousarray(
        np.asarray(Wq_w, np.float32).T.reshape(DT, 128, H).transpose(1, 0, 2))
    wkp = np.ascontiguousarray(
        np.asarray(Wk_w, np.float32).T.reshape(DT, 128, H).transpose(1, 0, 2))
    biasp = np.zeros((128, 8), np.float32)
    biasp[:, 0:4] = np.asarray(Wq_b, np.float32).reshape(HT, 128).T
    biasp[:, 4:8] = np.asarray(Wk_b, np.float32).reshape(HT, 128).T

    nc = _get_program()

    in_maps = []
    for core in range(N_CORES):
        b, h = divmod(core, 2)
        x_halo = np.zeros((SK, D), np.float32)
        if h == 0:
            x_halo[HALO:] = x[b, 0:HALF + HALO]
        else:
            x_halo[HALO:] = x[b, S - HALF - HALO:][::-1]
        # xTp[p, dt, c] = x_halo[c, dt*128+p]
        xTp = np.ascontiguousarray(
            x_halo.T.reshape(DT, 128, SK).transpose(1, 0, 2))
        # xrowp[p, kt, d] = x_halo[kt*128+p, d]
        xrp = np.ascontiguousarray(
            x_halo.reshape(KT, 128, D).transpose(1, 0, 2)).astype(bf16)
        in_maps.append({
            "xTp": xTp,
            "wqp": wqp,
            "wkp": wkp,
            "biasp": biasp,
            "xrowp": xrp,
        })

    res = run_bass_kernel_spmd(nc, in_maps, core_ids=list(range(N_CORES)),
                               trace=_trace)
    _cached["last_result"] = res

    y = np.zeros((B, S, D), np.float32)
    for core in range(N_CORES):
        b, h = divmod(core, 2)
        o = res.results[core]["out"].astype(np.float32)
        if h == 0:
            y[b, :HALF] = o
        else:
            y[b, HALF:] = o[::-1]
    return y


# revision 13
# speedup vs baseline: 1.5040x; 1.4359x over previous
"""Trainium2 Bass kernel for banded local attention (kernel_size=128).

Problem: x[4,4096,512]; q = x@Wq.T+bq, k = x@Wk.T+bk (H=512);
scores = q@k.T masked to |i-j|<128; softmax; out = attn @ x.

Sharding: 8 cores = 4 batches x 2 sequence halves (2048 queries each) with a
128-row halo of keys on each side (2304 local key rows, zero padded at the
global sequence edges). For the h=1 half the sequence is passed REVERSED so
the padded/invalid key region is always local rows [0,128) and the edge mask
is only needed for query block 0 -> all 8 cores run the identical program
(pure SPMD, no collectives). Host un-reverses the h=1 outputs.

Key algebraic fold (v4): s_ij = q_i.k_j = x_i^T (Wq^T Wk) x_j + (per-i const)
+ (Wk^T bq).x_j + (const). The per-i and const terms are softmax-invariant
and dropped; M = Wq^T Wk is folded on the host so the device projects ONLY
g = M x (the q-projection disappears entirely -- scores use raw xT, already
resident, as lhsT); beta_j = (Wk^T bq).x_j is added into the scores PSUM by
an all-ones matmul against a host-precomputed beta/128 tile.

Precision: the score spread (sigma ~ 22) makes the softmax near-argmax, so
x / M / g stay f32r (fp22); p / values / out are bf16 (p errors cancel via
the l normalizer; bf16 V adds ~0.2% per element).

Datapath per 128-query block:
  PSUM scores group: 4 f32r matmuls (xT-block^T @ gT window) + identity
  matmul adding the on-chip band mask + ones matmul adding beta. Then DVE
  negated rowmax -> ACT exp (bf16 p, row-sum l via accum_out; invalid
  positions exp(-1e30)=0) -> PE transposes (bf16 1 cyc/row) -> ACT cast to
  SBUF -> 3 bf16 matmuls against resident bf16 values -> DVE 1/l scale ->
  bf16 out DMA (host widens).
"""
import sys

if "/opt/trn_rl_repo" not in sys.path:
    sys.path.insert(0, "/opt/trn_rl_repo")

import numpy as np

B, S, D, H = 4, 4096, 512, 512
KS = 128
HALF = S // 2            # 2048 queries per core
HALO = KS                # 128
SK = HALF + 2 * HALO     # 2304 local key rows
KT = SK // 128           # 18 key row tiles
WIN = 3 * 128            # 384-wide key window per query block
NBLK = HALF // 128       # 16 query blocks
NEG = -1e30
N_CORES = 8

DT = D // 128   # 4 d-tiles

# g-projection chunks over SK columns (= xT chunk tiles); first chunk small
# so the first matmul starts as early as possible (f32r needs >= 256 free)
CHUNKS = [(0, 256), (256, 512), (768, 512), (1280, 512), (1792, 512)]
# attention blocks emitted once g covers the block's window
CHUNK_BLOCKS = [[], [0, 1, 2, 3], [4, 5, 6, 7], [8, 9, 10, 11],
                [12, 13, 14, 15]]

_cached = {}


def _build_program():
    import concourse.bass as bass
    import concourse.tile as tile
    import concourse.mybir as mybir
    from concourse import bacc
    from concourse.masks import make_identity

    f32 = mybir.dt.float32
    f32r = mybir.dt.float32r
    bf16 = mybir.dt.bfloat16
    AF = mybir.ActivationFunctionType
    AX = mybir.AxisListType

    nc = bacc.Bacc("TRN2", target_bir_lowering=False, debug=False,
                   num_devices=N_CORES)

    # dram inputs (host pre-arranged; see kernel())
    mt_d = nc.dram_tensor("mtp", [128, DT, D], f32r,
                          kind="ExternalInput").ap()
    xT_d = nc.dram_tensor("xTp", [128, DT, SK], f32r,
                          kind="ExternalInput").ap()
    beta_d = nc.dram_tensor("betap", [128, SK], bf16,
                            kind="ExternalInput").ap()
    xrow_d = nc.dram_tensor("xrowp", [128, KT, D], bf16,
                            kind="ExternalInput").ap()
    out_d = nc.dram_tensor("out", [HALF, D], bf16, kind="ExternalOutput").ap()

    with tile.TileContext(nc) as tc:
        with (
            tc.tile_pool(name="big", bufs=1) as big,
            tc.tile_pool(name="work", bufs=3) as work,
            tc.tile_pool(name="stat", bufs=6) as stat,
            tc.tile_pool(name="psP", bufs=2, space="PSUM") as psP,
            tc.tile_pool(name="psS", bufs=2, space="PSUM") as psS,
            tc.tile_pool(name="psT", bufs=2, space="PSUM") as psT,
            tc.tile_pool(name="psO", bufs=2, space="PSUM") as psO,
        ):
            # ---- input DMAs, ordered so g-proj chunk 0 starts earliest ----
            mt = big.tile([128, DT, D], f32r, tag="mt", name="mt")
            nc.sync.dma_start(mt, mt_d)
            xTc = [big.tile([128, DT, cw], f32r, tag=f"xT{c}", name=f"xT{c}")
                   for c, (c0, cw) in enumerate(CHUNKS)]
            for c, (c0, cw) in enumerate(CHUNKS[:2]):
                nc.sync.dma_start(xTc[c], xT_d[:, :, c0:c0 + cw])
            beta = big.tile([128, SK], bf16, tag="beta", name="beta")
            nc.sync.dma_start(beta, beta_d)
            for c in (2, 3, 4):
                c0, cw = CHUNKS[c]
                nc.sync.dma_start(xTc[c], xT_d[:, :, c0:c0 + cw])
            xrowp = big.tile([128, KT, D], bf16, tag="xrowp", name="xrowp")
            nc.sync.dma_start(xrowp, xrow_d)

            # ---- on-chip constants (GpSimd; overlaps the DMAs) ----
            ident = big.tile([128, 128], bf16, tag="ident")
            make_identity(nc, ident)
            ones = big.tile([128, 128], bf16, tag="ones")
            nc.gpsimd.memset(ones, 1.0)
            # additive band masks over the 384-wide window: tile 0 valid iff
            # y > a, tile 1 always valid, tile 2 valid iff y < a; the edge
            # variant (block 0) masks all of tile 0 (padded halo rows)
            masks = []
            for mi in range(2):
                mk = big.tile([128, WIN], bf16, tag=f"mask{mi}")
                nc.gpsimd.memset(mk, 0.0)
                if mi == 0:
                    nc.gpsimd.affine_select(
                        out=mk[:, 0:128], in_=mk[:, 0:128],
                        compare_op=mybir.AluOpType.is_ge, fill=NEG,
                        base=-1, pattern=[[1, 128]], channel_multiplier=-1)
                else:
                    nc.gpsimd.memset(mk[:, 0:128], NEG)
                nc.gpsimd.affine_select(
                    out=mk[:, 256:384], in_=mk[:, 256:384],
                    compare_op=mybir.AluOpType.is_ge, fill=NEG,
                    base=-1, pattern=[[-1, 128]], channel_multiplier=1)
                masks.append(mk)

            # ---- g projection: gT[dto][d, j] = sum_dti M x ----
            gT = [big.tile([128, SK], f32r, tag=f"gT{t}", name=f"gT{t}")
                  for t in range(DT)]
            ncopy = [0]

            def proj_chunk(c0, cw, xc):
                for dto in range(DT):
                    ps = psP.tile([128, 512], f32, tag="proj")
                    for dti in range(DT):
                        nc.tensor.matmul(
                            ps[:, :cw],
                            lhsT=mt[:, dti, dto * 128:(dto + 1) * 128],
                            rhs=xTc[xc][:, dti, 0:cw],
                            start=(dti == 0),
                            stop=(dti == DT - 1),
                        )
                    # alternate the PSUM->SBUF cast between ACT and DVE
                    if ncopy[0] % 2 == 0:
                        nc.scalar.activation(
                            gT[dto][:, c0:c0 + cw], ps[:, :cw],
                            AF.Identity, bias=0.0, scale=1.0)
                    else:
                        nc.vector.tensor_copy(
                            gT[dto][:, c0:c0 + cw], ps[:, :cw])
                    ncopy[0] += 1

            # ---- per query-block attention ----
            def attention(qb):
                j0 = qb * 128
                # query cols in xT: [HALO + j0, HALO + j0 + 128)
                qc = HALO + j0
                xc = next(c for c, (c0, cw) in enumerate(CHUNKS)
                          if c0 <= qc and qc + 128 <= c0 + cw)
                c0, cw = CHUNKS[xc]
                qoff = qc - c0
                s_ps = psS.tile([128, WIN], f32, tag="s")
                for dt_i in range(DT):
                    nc.tensor.matmul(
                        s_ps,
                        lhsT=xTc[xc][:, dt_i, qoff:qoff + 128],
                        rhs=gT[dt_i][:, j0:j0 + WIN],
                        start=(dt_i == 0),
                        stop=False,
                    )
                # band mask via identity matmul; beta via all-ones matmul
                # (sums 128 copies of beta/128)
                nc.tensor.matmul(
                    s_ps, lhsT=ident,
                    rhs=masks[1] if qb == 0 else masks[0],
                    start=False, stop=False)
                nc.tensor.matmul(
                    s_ps, lhsT=ones, rhs=beta[:, j0:j0 + WIN],
                    start=False, stop=True)

                negm = stat.tile([128, 1], f32, tag="negm")
                nc.vector.reduce_max(negm, s_ps, axis=AX.X, negate=True)
                p = work.tile([128, WIN], bf16, tag="p")
                lsum = stat.tile([128, 1], f32, tag="lsum")
                nc.scalar.activation(p, s_ps, AF.Exp, bias=negm, scale=1.0,
                                     accum_out=lsum)
                rinv = stat.tile([128, 1], f32, tag="rinv")
                nc.vector.reciprocal(rinv, lsum)

                # transpose p (bf16, 1 cyc/row); ACT copies PSUM -> SBUF
                pT_ps = psT.tile([128, 3, 128], bf16, tag="pT")
                for jt in range(3):
                    nc.tensor.transpose(
                        pT_ps[:, jt, :], p[:, jt * 128:(jt + 1) * 128], ident)
                pT = work.tile([128, 3, 128], bf16, tag="pTs")
                nc.scalar.activation(pT, pT_ps, AF.Identity, bias=0.0,
                                     scale=1.0)

                # out_blk[i, d] = sum_j p[i, j] xrow[j, d]
                o_ps = psO.tile([128, D], f32, tag="o")
                for jt in range(3):
                    nc.tensor.matmul(o_ps, lhsT=pT[:, jt, :],
                                     rhs=xrowp[:, qb + jt, :],
                                     start=(jt == 0), stop=(jt == 2))
                o_sb = work.tile([128, D], bf16, tag="o_sb")
                nc.vector.tensor_scalar_mul(o_sb, o_ps, rinv)
                nc.sync.dma_start(out_d[qb * 128:(qb + 1) * 128, :], o_sb)

            for c, (c0, cw) in enumerate(CHUNKS):
                proj_chunk(c0, cw, c)
                for qb in CHUNK_BLOCKS[c]:
                    attention(qb)

    nc.compile()
    return nc


def _get_program():
    if "nc" not in _cached:
        _cached["nc"] = _build_program()
    return _cached["nc"]


def kernel(x, Wq_w, Wq_b, Wk_w, Wk_b, _trace=False):
    import ml_dtypes
    from concourse.bass_utils import run_bass_kernel_spmd

    bf16 = ml_dtypes.bfloat16

    x = np.ascontiguousarray(np.asarray(x, np.float32))
    Wq_w = np.asarray(Wq_w, np.float64)
    Wk_w = np.asarray(Wk_w, np.float64)
    # fold the projections: s_ij = x_i^T M x_j + beta_j (+ softmax-invariant
    # terms); M = Wq^T Wk, v = Wk^T bq
    M = (Wq_w.T @ Wk_w).astype(np.float32)
    v = (Wk_w.T @ np.asarray(Wq_b, np.float64)).astype(np.float32)
    # mtp[p, dti, m] = M[m, dti*128+p]
    mtp = np.ascontiguousarray(
        M.T.reshape(DT, 128, D).transpose(1, 0, 2))

    nc = _get_program()

    in_maps = []
    for core in range(N_CORES):
        b, h = divmod(core, 2)
        x_halo = np.zeros((SK, D), np.float32)
        if h == 0:
            x_halo[HALO:] = x[b, 0:HALF + HALO]
        else:
            x_halo[HALO:] = x[b, S - HALF - HALO:][::-1]
        # xTp[p, dt, c] = x_halo[c, dt*128+p]
        xTp = np.ascontiguousarray(
            x_halo.T.reshape(DT, 128, SK).transpose(1, 0, 2))
        # xrowp[p, kt, d] = x_halo[kt*128+p, d]
        xrp = np.ascontiguousarray(
            x_halo.reshape(KT, 128, D).transpose(1, 0, 2)).astype(bf16)
        betar = ((x_halo @ v) / 128.0).astype(bf16)
        in_maps.append({
            "mtp": mtp,
            "xTp": xTp,
            "betap": np.ascontiguousarray(
                np.broadcast_to(betar[None, :], (128, SK))),
            "xrowp": xrp,
        })

    res = run_bass_kernel_spmd(nc, in_maps, core_ids=list(range(N_CORES)),
                               trace=_trace)
    _cached["last_result"] = res

    y = np.zeros((B, S, D), np.float32)
    for core in range(N_CORES):
        b, h = divmod(core, 2)
        o = res.results[core]["out"].astype(np.float32)
        if h == 0:
            y[b, :HALF] = o
        else:
            y[b, HALF:] = o[::-1]
    return y


# revision 20
# speedup vs baseline: 1.5429x; 1.0258x over previous
"""Trainium2 Bass kernel for banded local attention (kernel_size=128).

Problem: x[4,4096,512]; q = x@Wq.T+bq, k = x@Wk.T+bk (H=512);
scores = q@k.T masked to |i-j|<128; softmax; out = attn @ x.

Sharding: 8 cores = 4 batches x 2 sequence halves (2048 queries each) with a
128-row halo of keys on each side (2304 local key rows, zero padded at the
global sequence edges). For the h=1 half the sequence is passed REVERSED so
the padded/invalid key region is always local rows [0,128) and the edge mask
is only needed for query block 0 -> all 8 cores run the identical program
(pure SPMD, no collectives). Host un-reverses the h=1 outputs.

Key algebraic fold (v4): s_ij = q_i.k_j = x_i^T (Wq^T Wk) x_j + (per-i const)
+ (Wk^T bq).x_j + (const). The per-i and const terms are softmax-invariant
and dropped; M = Wq^T Wk is folded on the host so the device projects ONLY
g = M x (the q-projection disappears entirely -- scores use raw xT, already
resident, as lhsT); beta_j = (Wk^T bq).x_j is added into the scores PSUM by
an all-ones matmul against a host-precomputed beta/128 tile.

Precision: the score spread (sigma ~ 22) makes the softmax near-argmax, so
x / M / g stay f32r (fp22); p / values / out are bf16 (p errors cancel via
the l normalizer; bf16 V adds ~0.2% per element).

Datapath per 128-query block:
  PSUM scores group: 4 f32r matmuls (xT-block^T @ gT window) + identity
  matmul adding the on-chip band mask + ones matmul adding beta. Then DVE
  negated rowmax -> ACT exp (bf16 p, row-sum l via accum_out; invalid
  positions exp(-1e30)=0) -> PE transposes (bf16 1 cyc/row) -> ACT cast to
  SBUF -> 3 bf16 matmuls against resident bf16 values -> DVE 1/l scale ->
  bf16 out DMA (host widens).
"""
import sys

if "/opt/trn_rl_repo" not in sys.path:
    sys.path.insert(0, "/opt/trn_rl_repo")

import numpy as np

B, S, D, H = 4, 4096, 512, 512
KS = 128
HALF = S // 2            # 2048 queries per core
HALO = KS                # 128
SK = HALF + 2 * HALO     # 2304 local key rows
KT = SK // 128           # 18 key row tiles
WIN = 3 * 128            # 384-wide key window per query block
NBLK = HALF // 128       # 16 query blocks
NEG = -1e30
N_CORES = 8

DT = D // 128   # 4 d-tiles

# g-projection chunks over SK columns (= xT chunk tiles); first chunk small
# so the first matmul starts as early as possible (f32r needs >= 256 free);
# last chunks small so the final attention blocks start earlier
CHUNKS = [(0, 256), (256, 512), (768, 512), (1280, 512), (1792, 256),
          (2048, 256)]
# attention blocks emitted once g covers the block's window
CHUNK_BLOCKS = [[], [0, 1, 2, 3], [4, 5, 6, 7], [8, 9, 10, 11], [12, 13],
                [14, 15]]

_cached = {}


def _build_program():
    import concourse.bass as bass
    import concourse.tile as tile
    import concourse.mybir as mybir
    from concourse import bacc
    from concourse.masks import make_identity

    f32 = mybir.dt.float32
    f32r = mybir.dt.float32r
    bf16 = mybir.dt.bfloat16
    AF = mybir.ActivationFunctionType
    AX = mybir.AxisListType

    nc = bacc.Bacc("TRN2", target_bir_lowering=False, debug=False,
                   num_devices=N_CORES)

    # dram inputs (host pre-arranged; see kernel())
    mt_d = nc.dram_tensor("mtp", [128, DT, D], f32r,
                          kind="ExternalInput").ap()
    xT_d = nc.dram_tensor("xTp", [128, DT, SK], f32r,
                          kind="ExternalInput").ap()
    beta_d = nc.dram_tensor("betap", [128, SK], bf16,
                            kind="ExternalInput").ap()
    xrow_d = nc.dram_tensor("xrowp", [128, KT, D], bf16,
                            kind="ExternalInput").ap()
    out_d = nc.dram_tensor("out", [HALF, D], bf16, kind="ExternalOutput").ap()

    with tile.TileContext(nc) as tc:
        with (
            tc.tile_pool(name="big", bufs=1) as big,
            tc.tile_pool(name="work", bufs=3) as work,
            tc.tile_pool(name="stat", bufs=6) as stat,
            tc.tile_pool(name="psP", bufs=2, space="PSUM") as psP,
            tc.tile_pool(name="psS", bufs=2, space="PSUM") as psS,
            tc.tile_pool(name="psT", bufs=2, space="PSUM") as psT,
            tc.tile_pool(name="psO", bufs=2, space="PSUM") as psO,
        ):
            # ---- input DMAs, ordered so g-proj chunk 0 starts earliest:
            # M arrives in contraction-tile slices so the first accumulation
            # member only waits for slice 0 + the small first x chunk
            mt = [big.tile([128, D], f32r, tag=f"mt{i}", name=f"mt{i}")
                  for i in range(DT)]
            xTc = [big.tile([128, DT, cw], f32r, tag=f"xT{c}", name=f"xT{c}")
                   for c, (c0, cw) in enumerate(CHUNKS)]
            nc.sync.dma_start(mt[0], mt_d[:, 0, :])
            c0, cw = CHUNKS[0]
            nc.sync.dma_start(xTc[0], xT_d[:, :, c0:c0 + cw])
            for dti in range(1, DT):
                nc.sync.dma_start(mt[dti], mt_d[:, dti, :])
            c0, cw = CHUNKS[1]
            nc.sync.dma_start(xTc[1], xT_d[:, :, c0:c0 + cw])
            beta = big.tile([128, SK], bf16, tag="beta", name="beta")
            nc.sync.dma_start(beta, beta_d)
            for c in (2, 3, 4, 5):
                c0, cw = CHUNKS[c]
                nc.sync.dma_start(xTc[c], xT_d[:, :, c0:c0 + cw])
            xrowp = big.tile([128, KT, D], bf16, tag="xrowp", name="xrowp")
            nc.sync.dma_start(xrowp, xrow_d)

            # ---- on-chip constants (GpSimd; overlaps the DMAs) ----
            ident = big.tile([128, 128], bf16, tag="ident")
            make_identity(nc, ident)
            # additive band masks over the 384-wide window: tile 0 valid iff
            # y > a, tile 1 always valid, tile 2 valid iff y < a; the edge
            # variant (block 0) masks all of tile 0 (padded halo rows)
            masks = []
            for mi in range(2):
                mk = big.tile([128, WIN], bf16, tag=f"mask{mi}")
                nc.gpsimd.memset(mk, 0.0)
                if mi == 0:
                    nc.gpsimd.affine_select(
                        out=mk[:, 0:128], in_=mk[:, 0:128],
                        compare_op=mybir.AluOpType.is_ge, fill=NEG,
                        base=-1, pattern=[[1, 128]], channel_multiplier=-1)
                else:
                    nc.gpsimd.memset(mk[:, 0:128], NEG)
                nc.gpsimd.affine_select(
                    out=mk[:, 256:384], in_=mk[:, 256:384],
                    compare_op=mybir.AluOpType.is_ge, fill=NEG,
                    base=-1, pattern=[[-1, 128]], channel_multiplier=1)
                masks.append(mk)

            # ---- g projection: gT[dto][d, j] = sum_dti M x ----
            gT = [big.tile([128, SK], f32r, tag=f"gT{t}", name=f"gT{t}")
                  for t in range(DT)]
            ncopy = [0]

            def proj_chunk(c0, cw, xc):
                for dto in range(DT):
                    ps = psP.tile([128, 512], f32, tag="proj")
                    for dti in range(DT):
                        nc.tensor.matmul(
                            ps[:, :cw],
                            lhsT=mt[dti][:, dto * 128:(dto + 1) * 128],
                            rhs=xTc[xc][:, dti, 0:cw],
                            start=(dti == 0),
                            stop=(dti == DT - 1),
                        )
                    # alternate the PSUM->SBUF cast between ACT and DVE
                    if ncopy[0] % 2 == 0:
                        nc.scalar.activation(
                            gT[dto][:, c0:c0 + cw], ps[:, :cw],
                            AF.Identity, bias=0.0, scale=1.0)
                    else:
                        nc.vector.tensor_copy(
                            gT[dto][:, c0:c0 + cw], ps[:, :cw])
                    ncopy[0] += 1

            # ---- per query-block attention ----
            def attention(qb):
                j0 = qb * 128
                # query cols in xT: [HALO + j0, HALO + j0 + 128)
                qc = HALO + j0
                xc = next(c for c, (c0, cw) in enumerate(CHUNKS)
                          if c0 <= qc and qc + 128 <= c0 + cw)
                c0, cw = CHUNKS[xc]
                qoff = qc - c0
                # GpSimd (otherwise idle) combines band mask + beta window
                comb = work.tile([128, WIN], bf16, tag="comb")
                nc.gpsimd.tensor_add(
                    comb, masks[1] if qb == 0 else masks[0],
                    beta[:, j0:j0 + WIN])
                s_ps = psS.tile([128, WIN], f32, tag="s")
                for dt_i in range(DT):
                    nc.tensor.matmul(
                        s_ps,
                        lhsT=xTc[xc][:, dt_i, qoff:qoff + 128],
                        rhs=gT[dt_i][:, j0:j0 + WIN],
                        start=(dt_i == 0),
                        stop=False,
                    )
                # mask+beta added into the PSUM group via identity matmul
                nc.tensor.matmul(
                    s_ps, lhsT=ident, rhs=comb, start=False, stop=True)

                negm = stat.tile([128, 1], f32, tag="negm")
                nc.vector.reduce_max(negm, s_ps, axis=AX.X, negate=True)
                p = work.tile([128, WIN], bf16, tag="p")
                lsum = stat.tile([128, 1], f32, tag="lsum")
                nc.scalar.activation(p, s_ps, AF.Exp, bias=negm, scale=1.0,
                                     accum_out=lsum)
                rinv = stat.tile([128, 1], f32, tag="rinv")
                nc.vector.reciprocal(rinv, lsum)

                # transpose p (bf16, 1 cyc/row); ACT copies PSUM -> SBUF
                pT_ps = psT.tile([128, 3, 128], bf16, tag="pT")
                for jt in range(3):
                    nc.tensor.transpose(
                        pT_ps[:, jt, :], p[:, jt * 128:(jt + 1) * 128], ident)
                pT = work.tile([128, 3, 128], bf16, tag="pTs")
                nc.scalar.activation(pT, pT_ps, AF.Identity, bias=0.0,
                                     scale=1.0)

                # out_blk[i, d] = sum_j p[i, j] xrow[j, d]
                o_ps = psO.tile([128, D], f32, tag="o")
                for jt in range(3):
                    nc.tensor.matmul(o_ps, lhsT=pT[:, jt, :],
                                     rhs=xrowp[:, qb + jt, :],
                                     start=(jt == 0), stop=(jt == 2))
                o_sb = work.tile([128, D], bf16, tag="o_sb")
                nc.vector.tensor_scalar_mul(o_sb, o_ps, rinv)
                nc.sync.dma_start(out_d[qb * 128:(qb + 1) * 128, :], o_sb)

            for c, (c0, cw) in enumerate(CHUNKS):
                proj_chunk(c0, cw, c)
                for qb in CHUNK_BLOCKS[c]:
                    attention(qb)

    nc.compile()
    return nc


def _get_program():
    if "nc" not in _cached:
        _cached["nc"] = _build_program()
    return _cached["nc"]


def kernel(x, Wq_w, Wq_b, Wk_w, Wk_b, _trace=False):
    import ml_dtypes
    from concourse.bass_utils import run_bass_kernel_spmd

    bf16 = ml_dtypes.bfloat16

    x = np.ascontiguousarray(np.asarray(x, np.float32))
    Wq_w = np.asarray(Wq_w, np.float64)
    Wk_w = np.asarray(Wk_w, np.float64)
    # fold the projections: s_ij = x_i^T M x_j + beta_j (+ softmax-invariant
    # terms); M = Wq^T Wk, v = Wk^T bq
    M = (Wq_w.T @ Wk_w).astype(np.float32)
    v = (Wk_w.T @ np.asarray(Wq_b, np.float64)).astype(np.float32)
    # mtp[p, dti, m] = M[m, dti*128+p]
    mtp = np.ascontiguousarray(
        M.T.reshape(DT, 128, D).transpose(1, 0, 2))

    nc = _get_program()

    in_maps = []
    for core in range(N_CORES):
        b, h = divmod(core, 2)
        x_halo = np.zeros((SK, D), np.float32)
        if h == 0:
            x_halo[HALO:] = x[b, 0:HALF + HALO]
        else:
            x_halo[HALO:] = x[b, S - HALF - HALO:][::-1]
        # xTp[p, dt, c] = x_halo[c, dt*128+p]
        xTp = np.ascontiguousarray(
            x_halo.T.reshape(DT, 128, SK).transpose(1, 0, 2))
        # xrowp[p, kt, d] = x_halo[kt*128+p, d]
        xrp = np.ascontiguousarray(
            x_halo.reshape(KT, 128, D).transpose(1, 0, 2)).astype(bf16)
        betar = (x_halo @ v).astype(bf16)
        in_maps.append({
            "mtp": mtp,
            "xTp": xTp,
            "betap": np.ascontiguousarray(
                np.broadcast_to(betar[None, :], (128, SK))),
            "xrowp": xrp,
        })

    res = run_bass_kernel_spmd(nc, in_maps, core_ids=list(range(N_CORES)),
                               trace=_trace)
    _cached["last_result"] = res

    y = np.zeros((B, S, D), np.float32)
    for core in range(N_CORES):
        b, h = divmod(core, 2)
        o = res.results[core]["out"].astype(np.float32)
        if h == 0:
            y[b, :HALF] = o
        else:
            y[b, HALF:] = o[::-1]
    return y
